# revision 19
# baseline (speedup 1.0000x reference)
"""Trainium2 Bass kernel for GAP -> tiny Mamba (channel attention) -> broadcast multiply.

Reference computation (per batch):
    pooled = mean(x1 over H,W)                  # [C] ; sequence of length C=512, d_model=1
    att    = mamba(pooled)                      # d_inner=2, d_state=16, dt_rank=1, conv=4
    out    = x2 * att[None, None, :]

Sharding: data-parallel over batch B=16 across 8 cores (2 batches/core), params
replicated. Memory-bound: each core streams 16 MiB of x1 (GAP), 16 MiB of x2 in
and 16 MiB of out back (~50.3 MB -> ~117 us floor at the measured ~430 GB/s
aggregate DMA rate).

Schedule (per engine, program order):
  - Sync (HWDGE queue A): 16 x1 tile loads (1 MiB, batches interleaved), then
    8 x2 tile loads (2 MiB) into DEDICATED buffers (no reuse waits, so the
    queue never stalls on the mamba chain).
  - Scalar (HWDGE queue B): tiny a_col prep, the chain activations
    (Silu/Softplus/Exp, one op each), then the 8 output stores -- stores live
    on their own hardware queue so they start the moment a tile is multiplied.
  - Vector: batch-0 GAP accumulate (tracks the DMA stream), then the chain
    elementwise ops, then the 8 in-place broadcast multiplies.
  - Tensor: batch-1 GAP via 32 PSUM-accumulating ones-matmuls (parallel with
    Vector's batch-0 work), then the chain matmuls (x_proj B|C merged into one
    [4,128] stationary, dt|g merged into one [8,128] stationary).
  - GpSimd: consolidated weight scatters (~16 multi-dim-AP SWDGE DMAs instead
    of ~52 singles) + selector memsets; runs concurrently with the x1 stream.
"""

import os
import numpy as np

import concourse.bass as bass
import concourse.bacc as bacc
import concourse.tile as tile
from concourse import mybir
from concourse.bass_utils import run_bass_kernel_spmd

F32 = mybir.dt.float32
AF = mybir.ActivationFunctionType
OP = mybir.AluOpType

N_CORES = 8
B_FULL, H, W, C = 16, 64, 64, 512
B_LOC = B_FULL // N_CORES            # 2 batches per core
HW = H * W                           # 4096 spatial positions
# x1 streaming tiles: [128, Q1*C] (Q1 image rows per partition)
Q1 = 2
ROWS1 = 128 * Q1                     # 256 rows per tile
NT1 = HW // ROWS1                    # 16 tiles per batch image
# x2 streaming tiles: [128, Q2*C]
Q2 = 8
ROWS2 = 128 * Q2                     # 1024 rows per tile
NT2 = HW // ROWS2                    # 4 tiles per batch image

WEIGHT_SHAPES = {
    "in_proj_w": [4, 1],
    "conv_w": [2, 1, 4],
    "conv_b": [2],
    "x_proj_w": [33, 2],
    "dt_proj_w": [2, 1],
    "dt_proj_b": [2],
    "A_log": [2, 16],
    "Dp": [2],
    "out_proj_w": [1, 2],
}

LAST_RESULTS = None
_CACHE = {}


def _dap(handle, offset, pattern):
    return bass.AP(handle, offset, pattern)


def _build():
    # Bacc (not raw Bass): its compile() pipeline legalizes multi-wait
    # instructions, which this walrus version rejects on e.g. TensorTensor.
    nc = bacc.Bacc(None, target_bir_lowering=False, dynamic_dma_scratch_size=32768)

    x1h = nc.dram_tensor("x1", [B_LOC, H, W, C], F32, kind="ExternalInput")
    x2h = nc.dram_tensor("x2", [B_LOC, H, W, C], F32, kind="ExternalInput")
    wh = {
        name: nc.dram_tensor(name, shape, F32, kind="ExternalInput")
        for name, shape in WEIGHT_SHAPES.items()
    }
    outh = nc.dram_tensor("out", [B_LOC, H, W, C], F32, kind="ExternalOutput")

    # ---- inline 0/1 constants ----
    ones_col_d = nc.inline_tensor(np.full((128, 1), 1.0 / HW, np.float32), "c_ones_col")
    # u2[b] -> ubc rows (b, d)
    sel_u_np = np.zeros((33, 4), np.float32)
    for b in range(2):
        sel_u_np[32 * b, 2 * b : 2 * b + 2] = 1.0
    sel_u_d = nc.inline_tensor(sel_u_np, "c_sel_u")
    # dtg8 rows (b,d | 4+(b,d)) -> dtg64 rows (dt: 0..64, g: 64..128)
    bsel2_np = np.zeros((36, 128), np.float32)
    for r in range(4):
        bsel2_np[r, 16 * r : 16 * r + 16] = 1.0
        bsel2_np[32 + r, 64 + 16 * r : 64 + 16 * r + 16] = 1.0
    bsel2_d = nc.inline_tensor(bsel2_np, "c_bsel2")
    # (b,d,s) -> (b,d) reduction selector
    rsel_np = np.zeros((64, 4), np.float32)
    for r in range(64):
        rsel_np[r, r // 16] = 1.0
    rsel_d = nc.inline_tensor(rsel_np, "c_rsel")

    def img_ap1(handle, b, t):
        off = (b * HW + t * ROWS1) * C
        return _dap(handle, off, [[Q1 * C, 128], [1, Q1 * C]])

    def img_ap2(handle, b, t):
        off = (b * HW + t * ROWS2) * C
        return _dap(handle, off, [[Q2 * C, 128], [1, Q2 * C]])

    with tile.TileContext(nc) as tc:
        with (
            tc.tile_pool(name="x1pool", bufs=3) as x1pool,
            tc.tile_pool(name="x2pool", bufs=1) as x2pool,
            tc.tile_pool(name="work", bufs=1) as work,
            tc.tile_pool(name="psum", bufs=1, space="PSUM") as psum,
        ):
            # ============ setup: constants & weight-derived tiles (GpSimd) ====
            ones_col = work.tile([128, 1], F32)
            nc.gpsimd.dma_start(out=ones_col[:], in_=ones_col_d.ap())
            sel_u = work.tile([33, 4], F32)
            nc.gpsimd.dma_start(out=sel_u[:], in_=sel_u_d.ap())
            bsel2 = work.tile([36, 128], F32)
            nc.gpsimd.dma_start(out=bsel2[:], in_=bsel2_d.ap())
            rsel = work.tile([64, 4], F32)
            nc.gpsimd.dma_start(out=rsel[:], in_=rsel_d.ap())

            # per-(b,d) scalar columns, rows ordered r = 2*b + d; one DMA each
            winx_col = work.tile([4, 1], F32)
            wz_col = work.tile([4, 1], F32)
            convb_col = work.tile([4, 1], F32)
            dtw_col = work.tile([4, 1], F32)
            dtb_col = work.tile([4, 1], F32)
            dp_col = work.tile([4, 1], F32)
            nc.gpsimd.dma_start(out=winx_col[:], in_=_dap(wh["in_proj_w"], 0, [[0, 2], [1, 2]]))
            nc.gpsimd.dma_start(out=wz_col[:], in_=_dap(wh["in_proj_w"], 2, [[0, 2], [1, 2]]))
            nc.gpsimd.dma_start(out=convb_col[:], in_=_dap(wh["conv_b"], 0, [[0, 2], [1, 2]]))
            nc.gpsimd.dma_start(out=dtw_col[:], in_=_dap(wh["dt_proj_w"], 0, [[0, 2], [1, 2]]))
            nc.gpsimd.dma_start(out=dtb_col[:], in_=_dap(wh["dt_proj_b"], 0, [[0, 2], [1, 2]]))
            nc.gpsimd.dma_start(out=dp_col[:], in_=_dap(wh["Dp"], 0, [[0, 2], [1, 2]]))

            # conv taps (in_proj weight folded in later): wq[(b,d), j] = conv_w[d,0,j]
            wq = work.tile([4, 4], F32)
            nc.gpsimd.dma_start(out=wq[:], in_=_dap(wh["conv_w"], 0, [[0, 2], [4, 2], [1, 4]]))

            # x_proj composed selectors
            selDx = work.tile([4, 4], F32)     # dt_rank row -> rows (b,d)
            selBC = work.tile([4, 128], F32)   # cols 0:64 = B rows (b,d,s), 64:128 = C rows
            # out_proj broadcast stationaries: abp_b = oselbc[b]^T @ y4g
            oselbc0 = work.tile([4, 128], F32, tag="oselbc0")
            oselbc1 = work.tile([4, 128], F32, tag="oselbc1")
            oselbc = [oselbc0, oselbc1]
            dtg8 = work.tile([36, C], F32)
            nc.gpsimd.memset(dtg8[:], 0.0)
            nc.gpsimd.memset(selDx[:], 0.0)
            nc.gpsimd.memset(selBC[:], 0.0)
            nc.gpsimd.memset(oselbc[0][:], 0.0)
            nc.gpsimd.memset(oselbc[1][:], 0.0)
            for b in range(2):
                for d in range(2):
                    nc.gpsimd.dma_start(
                        out=selDx[2 * b : 2 * b + 2, 2 * b + d : 2 * b + d + 1],
                        in_=_dap(wh["x_proj_w"], 0, [[1, 2], [1, 1]]),
                    )
                for d in range(2):
                    nc.gpsimd.dma_start(
                        out=selBC[2 * b : 2 * b + 2, 32 * b + 16 * d : 32 * b + 16 * d + 16],
                        in_=_dap(wh["x_proj_w"], 2, [[1, 2], [2, 16], [1, 1]]),
                    )
                    nc.gpsimd.dma_start(
                        out=selBC[2 * b : 2 * b + 2, 64 + 32 * b + 16 * d : 64 + 32 * b + 16 * d + 16],
                        in_=_dap(wh["x_proj_w"], 34, [[1, 2], [2, 16], [1, 1]]),
                    )
                nc.gpsimd.dma_start(
                    out=oselbc[b][2 * b : 2 * b + 2, :],
                    in_=_dap(wh["out_proj_w"], 0, [[1, 2], [0, 128], [1, 1]]),
                )

            # A column [64, 1]: rows (b,d,s) = -exp(A_log[d, s]); one DMA + 2 scalar ops
            a_col = work.tile([64, 1], F32)
            nc.gpsimd.dma_start(
                out=a_col[:], in_=_dap(wh["A_log"], 0, [[0, 2], [16, 2], [1, 16]])
            )
            nc.scalar.activation(a_col[:], a_col[:], AF.Exp)
            dtwn_col = work.tile([4, 1], F32)
            dtbn_col = work.tile([4, 1], F32)
            nc.scalar.mul(dtwn_col[:], dtw_col[:], -1.0)
            nc.scalar.mul(dtbn_col[:], dtb_col[:], -1.0)

            # ============ phase 1: stream x1, GAP =============================
            # batch 0 accumulates on Vector; batch 1 accumulates on Tensor
            # (PSUM ones-matmuls) so both engines track the DMA stream.
            acc0 = work.tile([128, Q1 * C], F32, tag="acc0")
            gp = psum.tile([33, C], F32, tag="gp")
            gp1 = gp[32:33, :]
            for t in range(NT1):
                x1a = x1pool.tile([128, Q1 * C], F32, tag="x1a")
                nc.sync.dma_start(out=x1a[:], in_=img_ap1(x1h, 0, t))
                x1b = x1pool.tile([128, Q1 * C], F32, tag="x1b")
                nc.sync.dma_start(out=x1b[:], in_=img_ap1(x1h, 1, t))
                if t == 0:
                    nc.vector.tensor_copy(acc0[:], x1a[:])
                else:
                    nc.vector.tensor_add(acc0[:], acc0[:], x1a[:])
                for q in range(Q1):
                    nc.tensor.matmul(
                        gp1[:],
                        ones_col[:],
                        x1b[:, q * C : (q + 1) * C],
                        start=(t == 0 and q == 0),
                        stop=(t == NT1 - 1 and q == Q1 - 1),
                    )

            # x2 loads: dedicated buffers, issued right behind x1 on the same queue
            x2tiles = []
            for b in range(2):
                for t in range(NT2):
                    x2t = x2pool.tile([128, Q2 * C], F32, tag=f"x2_{b}_{t}")
                    nc.sync.dma_start(out=x2t[:], in_=img_ap2(x2h, b, t))
                    x2tiles.append(x2t)

            # fold in_proj into conv taps (Vector, after the accumulates)
            nc.vector.tensor_scalar_mul(wq[:], wq[:], winx_col[:])

            # GAP finish batch 0: tree-add then ones-matmul
            nc.vector.tensor_add(acc0[:, 0:512], acc0[:, 0:512], acc0[:, 512:1024])
            nc.tensor.matmul(gp[0:1, :], ones_col[:], acc0[:, 0:512], start=True, stop=True)

            u33 = work.tile([33, C], F32)
            nc.vector.tensor_copy(u33[0:1, :], gp[0:1, :])
            nc.vector.tensor_copy(u33[32:33, :], gp1)

            # ============ small mamba pipeline ================================
            ub8 = psum.tile([36, C], F32, tag="ub8")
            ubc = ub8[0:4, :]
            dtrbc = ub8[32:36, :]
            nc.tensor.matmul(ubc, sel_u[:], u33[:], start=True, stop=True)

            # causal depthwise conv (kernel 4) with folded input projection
            acc4 = work.tile([4, C], F32)
            nc.vector.tensor_scalar_mul(acc4[:], ubc[:], wq[:, 3:4])
            for j in (2, 1, 0):
                s = 3 - j
                nc.vector.scalar_tensor_tensor(
                    acc4[:, s:C], ubc[:, 0 : C - s], wq[:, j : j + 1], acc4[:, s:C],
                    op0=OP.mult, op1=OP.add,
                )
            xconv4 = work.tile([4, C], F32)
            nc.scalar.activation(xconv4[:], acc4[:], AF.Silu, bias=convb_col[:])
            # silu(z) with z = u * w_in[2+d]  (zpre reuses acc4's buffer)
            nc.vector.tensor_scalar_mul(acc4[:], ubc[:], wz_col[:])
            sz4 = work.tile([4, C], F32)
            nc.scalar.activation(sz4[:], acc4[:], AF.Silu)

            # x_proj slices via composed selectors
            nc.tensor.matmul(dtrbc, selDx[:], xconv4[:], start=True, stop=True)
            bc128 = psum.tile([128, C], F32, tag="bc128")
            nc.tensor.matmul(bc128[:], selBC[:], xconv4[:], start=True, stop=True)
            bm64 = work.tile([64, C], F32)
            nc.vector.tensor_copy(bm64[:], bc128[0:64, :])

            # dt = softplus(dtr * dt_proj_w + dt_proj_b); g = dt * xconv
            nc.scalar.activation(
                dtg8[0:4, :], dtrbc, AF.Sigmoid, bias=dtbn_col[:], scale=dtwn_col[:]
            )
            nc.scalar.activation(dtg8[0:4, :], dtg8[0:4, :], AF.Ln)
            nc.vector.scalar_tensor_tensor(
                dtg8[32:36, :], dtg8[0:4, :], -1.0, xconv4[:], op0=OP.mult, op1=OP.mult
            )

            dtg64 = psum.tile([128, C], F32, tag="dtg64")
            nc.tensor.matmul(dtg64[:], bsel2[:], dtg8[:], start=True, stop=True)

            # dA = exp(dt * A); dBu = (dt*x) * B   on 64 (b,d,s) lanes
            da64 = work.tile([64, C], F32)
            nc.scalar.activation(da64[:], dtg64[0:64, :], AF.Exp, scale=a_col[:])
            dbu64 = work.tile([64, C], F32)
            nc.vector.tensor_mul(dbu64[:], dtg64[64:128, :], bm64[:])

            # selective scan: h[:, t] = dA[:, t]*h[:, t-1] + dBu[:, t]
            h64 = work.tile([64, C], F32)
            nc.vector.tensor_tensor_scan(
                h64[:], da64[:], dbu64[:], 0.0, op0=OP.mult, op1=OP.add
            )

            # y = C . h (reduce s), + D*x, * silu(z), out_proj
            # (hc64 reuses da64's buffer; y4g reuses dtg8[0:4])
            nc.vector.tensor_mul(da64[:], h64[:], bc128[64:128, :])
            ya6 = psum.tile([4, C], F32, tag="ya6")
            y4p = ya6[0:4, :]
            nc.tensor.matmul(y4p, rsel[:], da64[:], start=True, stop=True)
            y4g = dtg8[0:4, :]
            nc.vector.scalar_tensor_tensor(
                y4g, xconv4[:], dp_col[:], y4p, op0=OP.mult, op1=OP.add
            )
            nc.vector.tensor_mul(y4g, y4g, sz4[:])

            # att[b] = sum_d out_proj_w[0,d] * y[b,d], broadcast to 128 rows in
            # ONE matmul per batch straight from y4g; phase 2 reads PSUM directly
            att_bc = []
            att_sb = []
            for b in range(2):
                abp = psum.tile([128, C], F32, tag=f"abp{b}")
                nc.tensor.matmul(abp[:], oselbc[b][:], y4g, start=True, stop=True)
                att_bc.append(abp)
            # SBUF copies for the GpSimd multiplies (GpSimd cannot read PSUM);
            # done on the Scalar engine, parallel with Vector's first multiply
            for b in range(2):
                asb = work.tile([128, C], F32, tag=f"attsb{b}")
                nc.scalar.activation(asb[:], att_bc[b][:], AF.Copy)
                att_sb.append(asb)

            # ============ phase 2: x2 * att -> out (in place) =================
            # multiplies on Vector (6 tiles) + GpSimd (2 tiles, stored last);
            # stores on the Scalar HWDGE queue
            def bcast_ap(b):
                ab = att_bc[b]
                return bass.AP(ab.tensor, ab.offset, [ab.ap[0], [0, Q2], [1, C]])

            gp_tiles = [(0, 1), (1, 1)]
            for b in range(2):
                for t in range(NT2):
                    if (b, t) in gp_tiles:
                        continue
                    x2t = x2tiles[b * NT2 + t]
                    v = x2t.rearrange("p (q c) -> p q c", q=Q2)
                    nc.vector.tensor_mul(v, v, bcast_ap(b))
                    nc.scalar.dma_start(out=img_ap2(outh, b, t), in_=x2t[:])
            for b, t in gp_tiles:
                x2t = x2tiles[b * NT2 + t]
                v = x2t.rearrange("p (q c) -> p q c", q=Q2)
                asb = att_sb[b]
                bcs = bass.AP(asb.tensor, asb.offset, [asb.ap[0], [0, Q2], [1, C]])
                nc.gpsimd.tensor_mul(v, v, bcs)
                nc.scalar.dma_start(out=img_ap2(outh, b, t), in_=x2t[:])

    nc.compile()
    return nc


def _get_nc():
    if "nc" not in _CACHE:
        _CACHE["nc"] = _build()
    return _CACHE["nc"]


def kernel(**inputs):
    global LAST_RESULTS
    nc = _get_nc()
    ins = {k: np.ascontiguousarray(np.asarray(v, dtype=np.float32)) for k, v in inputs.items()}

    in_maps = []
    for i in range(N_CORES):
        m = {name: ins[name] for name in WEIGHT_SHAPES}
        m["x1"] = np.ascontiguousarray(ins["x1"][B_LOC * i : B_LOC * (i + 1)])
        m["x2"] = np.ascontiguousarray(ins["x2"][B_LOC * i : B_LOC * (i + 1)])
        in_maps.append(m)

    res = run_bass_kernel_spmd(
        nc,
        in_maps,
        core_ids=list(range(N_CORES)),
        trace=bool(int(os.environ.get("BASS_TRACE", "0") or "0")),
    )
    LAST_RESULTS = res
    return np.concatenate([r["out"] for r in res.results], axis=0)
